# revision 22
# baseline (speedup 1.0000x reference)
"""MoE router kernel for Trainium2 (8 NeuronCores, Bass/Tile).

Computation per token t (32768 tokens, H=1024, E=8, K=2):
  logits = h @ W.T ; probs = softmax(logits) ; top-2 (values renormalized,
  indices int32) ; aux partials (per-expert prob sums; counts finished on host
  from the returned indices).

Sharding: data-parallel over flattened tokens — core c handles tokens
[c*4096, (c+1)*4096). Gate weight replicated. Host does the tiny aux-loss
reduction from per-core partials.

Per-core dataflow:
  token mapping t = p*32 + g (p = SBUF partition, g = block) so that both the
  input loads and output stores are contiguous per partition.
  1. DMA [128, 1024] fp32 tiles (512 KB, fully coalesced).
  2. PE transposes each [128t, 128h] chunk (is_transpose, float32r view) into
     PSUM; DVE/ACT copy chunks to SBUF as hT [128h, tokens].
  3. Gate matmul with pre-transposed W chunk [128h, 8e] stationary and hT
     moving (N=256 tokens per instruction, float32r) -> logitsT [8, 256] PSUM.
  4. logitsT -> SBUF -> tiny PE hop-transpose back to [128t, 8e] -> one PSUM
     tile [128, 256] holding all 4096 logits of the core.
  5. DVE softmax over the free dim (segments of 8), Max8 + MaxIndex8 for the
     sorted top-2 values+indices, renormalize, emit outputs.
"""

import sys

if "/opt/trn_rl_repo" not in sys.path:
    sys.path.insert(0, "/opt/trn_rl_repo")

import numpy as np

import concourse.bass as bass
import concourse.mybir as mybir
from concourse.masks import make_identity
from concourse.tile import TileContext
from concourse.vector_clock import ScopedClock, VectorClock
import concourse.tile as _tile_mod


def _split_drain_and_barrier(self, tick_clock, wait_clock):
    # The stock kernel-tail drain carries one wait per live semaphore (~15),
    # which exceeds this codegen's per-instruction sync-wait limit. Emit one
    # drain per proc instead, each with a single wait.
    gc = tick_clock.global_clock
    vec = list(gc)
    for i, v in enumerate(vec):
        if v == 0:
            continue
        sub = [0] * len(vec)
        sub[i] = v
        d = self.nc.sync.drain()
        wait_clock.add_sem_waits(d.ins, ScopedClock({None: VectorClock(sub)}))
    self.nc.all_engine_barrier()
    popped = self.nc._tile_sem_poison_stack.pop()
    assert popped is self._sem_poison
    self.nc.clear_and_free_semaphores(list(self.sems.allocated().values()))
    self.nc.all_engine_barrier()


_tile_mod.TileContext._drain_and_barrier = _split_drain_and_barrier

NCORES = 8
B, S, H = 4, 8192, 1024
E = 8
TOPK = 2
T = B * S                  # 32768 tokens
TPC = T // NCORES          # 4096 tokens per core
P = 128                    # partitions
HC = H // P                # 8 h-chunks
G = TPC // P               # 32 token blocks per core
PAIR = 2                   # token blocks per gate matmul (N = 256)

# float32r streams fp32 through the PE at 1 cycle/row (vs 4 for float32) when
# the moving dim is >= 256. The BIR verifier requires f32r matmul operands to
# be produced by instructions that round to f32r, so the PSUM->SBUF copies
# write f32r. Transposes stay plain fp32 (bit-exact data movement).
MM_DT = mybir.dt.float32r

F32 = mybir.dt.float32
I32 = mybir.dt.int32
U32 = mybir.dt.uint32
AX = mybir.AxisListType
OP = mybir.AluOpType
ACTF = mybir.ActivationFunctionType


def build_bass():
    nc = bass.Bass(target_bir_lowering=True)

    h_in = nc.dram_tensor("h", [TPC, H], F32, kind="ExternalInput")
    # wt[p, hc*8+e] = W[e, hc*128+p] (host pre-transposes the tiny gate weight)
    wt_in = nc.dram_tensor("wt", [P, HC * E], F32, kind="ExternalInput")
    w_out = nc.dram_tensor("w_out", [P, G * TOPK], F32, kind="ExternalOutput")
    i_out = nc.dram_tensor("i_out", [P, G * TOPK], I32, kind="ExternalOutput")
    p_out = nc.dram_tensor("p_out", [P, E], F32, kind="ExternalOutput")

    with TileContext(nc) as tc:
        with (
            tc.tile_pool(name="consts", bufs=1) as consts,
            tc.tile_pool(name="loads", bufs=1) as loads,
            tc.tile_pool(name="hts", bufs=2) as hts,
            tc.tile_pool(name="lgt", bufs=2) as lgt,
            tc.tile_pool(name="sm", bufs=1) as sm,
            tc.tile_pool(name="pst", bufs=2, space="PSUM") as pst,
            tc.tile_pool(name="pslg", bufs=2, space="PSUM") as pslg,
            tc.tile_pool(name="psl", bufs=1, space="PSUM") as psl,
        ):
            ident = consts.tile([P, P], F32)
            make_identity(nc, ident)

            wt_sb = consts.tile([P, HC * E], F32)
            nc.sync.dma_start(wt_sb[:, :], wt_in[:, :])
            wt_r = consts.tile([P, HC * E], MM_DT)
            nc.vector.tensor_copy(wt_r[:, :], wt_sb[:, :])

            # per-engine scratch for wait-absorber reads
            junk_v = consts.tile([P, 1], F32)
            junk_s = consts.tile([P, 1], F32)

            # all 4096 logits of this core, [128t, 32g * 8e]
            ps_logits = psl.tile([P, G * E], F32)

            # Dummy transpose: absorbs the gpsimd(identity) dependency into
            # PE's vector clock so real transposes carry only their DMA wait
            # (fp32 self-loading Matmult supports a single sync wait).
            nc.tensor.transpose(
                ps_logits[0:E, 0:E], ident[0:E, 0:E], ident[0:E, 0:E]
            )

            # DRAM view: block g, partition p -> token row p*G + g
            h_blocks = h_in[:, :].rearrange("(p g) d -> g p d", g=G)

            # DRAM view for one-DMA-per-pair loads: [p, g, d]
            h_pg = h_in[:, :].rearrange("(p g) d -> p g d", g=G)

            for pr in range(G // PAIR):
                g0 = PAIR * pr
                # hT chunks for both blocks: [128h, hc, block, 128t].
                # Two tiles (DVE- and ACT-written) so slot reuse of one never
                # makes the other engine's copy wait on both engines.
                hT_a = hts.tile([P, 4 * PAIR * P], MM_DT, tag="hTA")
                hT_b = hts.tile([P, 4 * PAIR * P], MM_DT, tag="hTB")
                hTa3 = hT_a[:, :].rearrange("p (c s t) -> p c s t", c=4, s=PAIR)
                hTb3 = hT_b[:, :].rearrange("p (c s t) -> p c s t", c=4, s=PAIR)

                # one 1 MB load per pair into a dedicated tile (no slot reuse
                # -> the DMA never needs a PE wait on top of its lane wait)
                h_L = loads.tile([P, PAIR * H], F32, tag=f"h{pr}")
                nc.sync.dma_start(
                    h_L[:, :].rearrange("p (s d) -> p s d", s=PAIR),
                    h_pg[:, g0 : g0 + PAIR, :],
                )

                # chunk-major PSUM staging: [chunk, block, 128t], 2 banks each
                ps_a = pst.tile([P, 4 * PAIR * P], F32, tag="psT")
                ps_b = pst.tile([P, 4 * PAIR * P], F32, tag="psT")
                # dummy transposes absorb the PSUM-slot-reuse wait so each
                # real transpose carries only its DMA wait (fp32 Matmult is
                # limited to a single sync wait in this codegen)
                nc.tensor.transpose(ps_a[:, 0:E], ident[0:E, :], ident[0:E, 0:E])
                nc.tensor.transpose(ps_b[:, 0:E], ident[0:E, :], ident[0:E, 0:E])
                for hc in range(HC):
                    dst = ps_a if hc < 4 else ps_b
                    for s in range(PAIR):
                        col = (hc % 4) * PAIR + s
                        nc.tensor.transpose(
                            dst[:, col * P : (col + 1) * P],
                            h_L[:, (s * HC + hc) * P : (s * HC + hc + 1) * P],
                            ident[:, :],
                        )
                # PSUM -> SBUF with f32r rounding; one producer per mm operand.
                # Absorber reads of the last-written element move the PE wait
                # onto a multi-wait-capable instruction first, so the rounding
                # copies themselves carry at most one wait.
                nc.vector.tensor_copy(junk_v[:, 0:1], ps_a[:, 1023:1024])
                nc.vector.tensor_copy(hT_a[:, :], ps_a[:, :])
                nc.scalar.copy(junk_s[:, 0:1], ps_b[:, 1023:1024])
                nc.scalar.copy(hT_b[:, :], ps_b[:, :])

                # gate matmul: accumulate over h-chunks, moving N = 256 tokens
                ps_lg = pslg.tile([E, PAIR * P], F32, tag="lgT")
                nc.tensor.transpose(
                    ps_lg[0:E, 0:E], ident[0:E, 0:E], ident[0:E, 0:E]
                )
                for hc in range(HC):
                    rhs = hTa3[:, hc, :, :] if hc < 4 else hTb3[:, hc - 4, :, :]
                    nc.tensor.matmul(
                        ps_lg[:, :],
                        lhsT=wt_r[:, hc * E : (hc + 1) * E],
                        rhs=rhs,
                        start=(hc == 0),
                        stop=(hc == HC - 1),
                    )

                sb_lg = lgt.tile([E, PAIR * P], F32, tag="sblg")
                nc.scalar.copy(junk_s[0:E, 0:1], ps_lg[:, 255:256])
                nc.scalar.copy(sb_lg[:, :], ps_lg[:, :])

                # hop back to [128t, 8e]
                for s in range(PAIR):
                    g = g0 + s
                    nc.tensor.transpose(
                        ps_logits[:, g * E : (g + 1) * E],
                        sb_lg[:, s * P : (s + 1) * P],
                        ident[0:E, 0:E],
                    )

            # ---- softmax + top-2, all 4096 tokens at once ----
            lg = sm.tile([P, G * E], F32)
            nc.vector.tensor_copy(lg[:, :], ps_logits[:, :])
            lg3 = lg[:, :].rearrange("p (g e) -> p g e", e=E)

            m1 = sm.tile([P, G], F32)
            nc.vector.tensor_reduce(m1[:, :], lg3, axis=AX.X, op=OP.max)
            m1b = m1[:, :].unsqueeze(2).to_broadcast([P, G, E])

            xs = sm.tile([P, G * E], F32)
            xs3 = xs[:, :].rearrange("p (g e) -> p g e", e=E)
            nc.vector.tensor_tensor(xs3, lg3, m1b, op=OP.subtract)

            ex = sm.tile([P, G * E], F32)
            nc.scalar.activation(ex[:, :], xs[:, :], ACTF.Exp)
            ex3 = ex[:, :].rearrange("p (g e) -> p g e", e=E)

            ssum = sm.tile([P, G], F32)
            nc.vector.tensor_reduce(ssum[:, :], ex3, axis=AX.X, op=OP.add)
            rsum = sm.tile([P, G], F32)
            nc.vector.reciprocal(rsum[:, :], ssum[:, :])
            rb = rsum[:, :].unsqueeze(2).to_broadcast([P, G, E])

            probs = sm.tile([P, G * E], F32)
            probs3 = probs[:, :].rearrange("p (g e) -> p g e", e=E)
            nc.vector.tensor_tensor(probs3, ex3, rb, op=OP.mult)

            # per-expert prob partial sums (reduce over g via transposed view)
            p8 = sm.tile([P, E], F32)
            nc.vector.tensor_reduce(
                p8[:, :], probs[:, :].rearrange("p (g e) -> p e g", e=E),
                axis=AX.X, op=OP.add,
            )
            nc.gpsimd.dma_start(p_out[:, :], p8[:, :])

            w_t = sm.tile([P, G * TOPK], F32)
            i_t = sm.tile([P, G * TOPK], I32)

            for g in range(G):
                pg = probs[:, g * E : (g + 1) * E]
                mx = sm.tile([P, 8], F32, tag="mx")
                mi = sm.tile([P, 8], U32, tag="mi")
                nc.vector.max(out=mx[:, :], in_=pg)
                nc.vector.max_index(out=mi[:, :], in_max=mx[:, :], in_values=pg)

                ws = sm.tile([P, 1], F32, tag="ws")
                nc.vector.tensor_tensor(
                    ws[:, :], mx[:, 0:1], mx[:, 1:2], op=OP.add
                )
                rw = sm.tile([P, 1], F32, tag="rw")
                nc.vector.reciprocal(rw[:, :], ws[:, :])
                nc.vector.tensor_scalar(
                    w_t[:, g * TOPK : (g + 1) * TOPK],
                    mx[:, 0:TOPK],
                    rw[:, 0:1],
                    None,
                    op0=OP.mult,
                )
                nc.vector.tensor_copy(i_t[:, g * TOPK : (g + 1) * TOPK], mi[:, 0:TOPK])

            nc.gpsimd.dma_start(w_out[:, :], w_t[:, :])
            nc.gpsimd.dma_start(i_out[:, :], i_t[:, :])

    return nc


_NC_CACHE = None


def _get_nc():
    global _NC_CACHE
    if _NC_CACHE is None:
        _NC_CACHE = build_bass()
    return _NC_CACHE


def kernel(hidden_states, W_gate):
    from concourse.bass_utils import run_bass_kernel_spmd

    hs = np.ascontiguousarray(np.asarray(hidden_states, dtype=np.float32)).reshape(T, H)
    W = np.asarray(W_gate, dtype=np.float32)
    wt = np.ascontiguousarray(
        W.reshape(E, HC, P).transpose(2, 1, 0).reshape(P, HC * E)
    )

    nc = _get_nc()
    in_maps = [
        {"h": np.ascontiguousarray(hs[c * TPC : (c + 1) * TPC]), "wt": wt}
        for c in range(NCORES)
    ]
    res = run_bass_kernel_spmd(nc, in_maps, core_ids=list(range(NCORES)))
    outs = res.results

    w_full = np.empty((T, TOPK), np.float32)
    i_full = np.empty((T, TOPK), np.int32)
    psum = np.zeros(E, np.float64)
    for c in range(NCORES):
        w_full[c * TPC : (c + 1) * TPC] = outs[c]["w_out"].reshape(TPC, TOPK)
        i_full[c * TPC : (c + 1) * TPC] = outs[c]["i_out"].reshape(TPC, TOPK)
        psum += outs[c]["p_out"].astype(np.float64).sum(axis=0)

    counts = np.bincount(i_full.reshape(-1), minlength=E).astype(np.float64)
    tokens_per_expert = counts / T
    router_prob_per_expert = psum / T
    aux_loss = np.float32(E * np.sum(tokens_per_expert * router_prob_per_expert))

    routing_weights = w_full.reshape(B, S, TOPK, 1)
    selected_experts = i_full.reshape(B, S, TOPK)
    return (routing_weights, selected_experts, aux_loss)
